# revision 29
# baseline (speedup 1.0000x reference)
"""Trainium2 Bass kernel for CustomMamba (data-parallel over (b*n) scans).

Self-contained: builds + compiles a single-core SPMD Bass/Tile program,
shards inputs over 8 NeuronCores (along n), runs via run_bass_kernel_spmd,
and gathers the full output.

v2: debug/asserts off, bf16 scan-phase elementwise (DVE 2x mode), scan op
on GpSimd (frees DVE), merged d-halves (DH=128), activation table-set
batching (Silu + natural_log_exp only), no DRAM spills, batched out-DMA.
"""

import sys

sys.path.insert(0, "/opt/trn_rl_repo")

import os

os.environ.setdefault("JAX_PLATFORMS", "cpu")

from contextlib import ExitStack

import numpy as np

import concourse.bacc as bacc
import concourse.mybir as mybir
from concourse.bass_utils import run_bass_kernel_spmd
from concourse.masks import make_identity
from concourse.tile import TileContext
from concourse.tile_rust import add_dep_helper

FP = mybir.dt.float32
BF = mybir.dt.bfloat16
AF = mybir.ActivationFunctionType
OP = mybir.AluOpType

# Problem constants (hardcoded per spec)
B, T, N, F = 8, 24, 512, 64
DI, DS, DR, DC = 128, 16, 4, 4
NCORES = 8


def _host_consts(inputs):
    """Fold the linear layers into per-stage weight matrices (fp32 numpy)."""
    w_mix = np.asarray(inputs["w_mix"], np.float32)      # [2F, F]
    b_mix = np.asarray(inputs["b_mix"], np.float32)      # [F]
    w_in = np.asarray(inputs["w_in"], np.float32)        # [F, 2*DI]
    conv_w = np.asarray(inputs["conv_w"], np.float32)    # [DI, DC]
    conv_b = np.asarray(inputs["conv_b"], np.float32)    # [DI]
    w_xproj = np.asarray(inputs["w_xproj"], np.float32)  # [DI, DR+2*DS]
    w_dt = np.asarray(inputs["w_dt"], np.float32)        # [DR, DI]
    b_dt = np.asarray(inputs["b_dt"], np.float32)        # [DI]
    A_log = np.asarray(inputs["A_log"], np.float32)      # [DI, DS]
    D = np.asarray(inputs["D"], np.float32)              # [DI]
    w_out = np.asarray(inputs["w_out"], np.float32)      # [DI, F]

    W1 = w_mix @ w_in                                    # [128, 2*DI]
    b1 = b_mix @ w_in                                    # [2*DI]
    W1x, W1z = W1[:, :DI].copy(), W1[:, DI:].copy()
    b1x, b1z = b1[:DI].copy(), b1[DI:].copy()
    # conv folded into M1: W1xk[k] = W1x * conv_w[:, k] (per-output-column)
    W1xk = [np.ascontiguousarray(W1x * conv_w[None, :, k]) for k in range(DC)]

    W2dt = (w_xproj[:, :DR] @ w_dt).copy()               # [DI, DI]
    W2bc = w_xproj[:, DR:].copy()                        # [DI, 2*DS]

    A = -np.exp(A_log)                                   # [DI, DS]
    assert np.allclose(A, A[0:1, :], rtol=1e-6), "A varies across d"
    A_s = [float(A[0, s]) for s in range(DS)]

    return dict(
        W1x=W1x, W1z=W1z, b1x=b1x, b1z=b1z, W1xk=W1xk,
        W2dt=W2dt, W2bc=W2bc, b_dt=b_dt,
        conv_w=conv_w, conv_b=conv_b, D=D, w_out=w_out, A_s=A_s,
    )


def build_program(n_c, consts, scan_eng="vector", x_eng="gpsimd",
                  cmul_eng="vector", add_eng="vector", n_dve_scan=0):
    """Build + compile the per-core Bass program. n_c = n-shard width."""
    nc = bacc.Bacc(
        "TRN2",
        target_bir_lowering=False,
        debug=False,
        enable_asserts=False,
        num_devices=1,
    )

    bn = B * n_c
    ic = min(128, bn)
    nblk = bn // ic
    assert nblk * ic == bn
    bpb = ic // n_c                    # b's per block
    assert bpb * n_c == ic and bpb >= 1
    CT = ic * T

    x_d = nc.dram_tensor("x_sh", (B, T, n_c, F), FP, kind="ExternalInput").ap()
    qk_d = nc.dram_tensor("qk_sh", (B, T, n_c, F), FP, kind="ExternalInput").ap()
    cd = {}
    for nm, shp in [
        ("W1z", (2 * F, DI)),
        ("W1k0", (2 * F, DI)), ("W1k1", (2 * F, DI)),
        ("W1k2", (2 * F, DI)), ("W1k3", (2 * F, DI)),
        ("W2dt", (DI, DI)), ("W2bc", (DI, 2 * DS)), ("b_dt", (DI, 1)),
        ("D", (DI, 1)), ("w_out", (DI, F)),
    ]:
        cd[nm] = nc.dram_tensor(nm, shp, FP, kind="ExternalInput").ap()
    out_d = nc.dram_tensor("out_sh", (B, T, n_c, F), FP, kind="ExternalOutput").ap()

    with TileContext(nc) as tc:
        _body(nc, tc, x_d, qk_d, cd, out_d,
              n_c, ic, nblk, bpb, CT, consts,
              scan_eng, x_eng, cmul_eng, add_eng, n_dve_scan)
    nc.compile()
    return nc


def _body(nc, tc, x_d, qk_d, cd, out_d,
          n_c, ic, nblk, bpb, CT, consts,
          scan_eng, x_eng, cmul_eng, add_eng, n_dve_scan):
    P = ic
    NMM = 512 if CT % 512 == 0 else CT  # matmul N-chunk
    TG = 4                              # t's merged per transpose-psum tile
    use_b1 = not (np.allclose(consts["b1x"], 0) and np.allclose(consts["b1z"], 0))
    use_cb = not np.allclose(consts["conv_b"], 0)
    A_s = consts["A_s"]

    es = ExitStack()
    sb = es.enter_context(tc.tile_pool(name="sb", bufs=1))
    sb2 = es.enter_context(tc.tile_pool(name="sb2", bufs=2))
    ps = es.enter_context(tc.tile_pool(name="ps", bufs=2, space="PSUM"))

    # ---- constants ----
    ct = {}
    for nm in cd:
        t = sb.tile(list(cd[nm].shape), FP, tag=f"c_{nm}")
        nc.sync.dma_start(t[:], cd[nm])
        ct[nm] = t
    w_out_bf = sb.tile([DI, F], BF, tag="c_w_out_bf")
    nc.vector.tensor_copy(out=w_out_bf[:], in_=ct["w_out"][:])
    ident = sb.tile([128, 128], FP, tag="ident")
    make_identity(nc, ident[:])
    ident_bf = sb.tile([128, 128], BF, tag="ident_bf")
    nc.vector.tensor_copy(out=ident_bf[:], in_=ident[:])

    BSUB = min(bpb, 128 // T)          # b's per load/transpose chunk

    scan_p = nc.gpsimd if scan_eng == "gpsimd" else nc.vector
    xeng = nc.vector if x_eng == "vector" else nc.gpsimd
    ceng = nc.vector if cmul_eng == "vector" else nc.gpsimd
    aeng = nc.vector if add_eng == "vector" else nc.gpsimd

    assert not use_b1 and not use_cb, "conv-folded M1 assumes zero biases"
    TP = T + DC - 1                    # padded t-stride (27)
    CTP = P * TP
    IML = 16                           # i's per M1 chunk
    CML = IML * TP                     # M1 chunk cols (432)

    act_prev = [None]

    def chain(bi):
        """Force ACT instruction order for table-set batching."""
        if act_prev[0] is not None:
            add_dep_helper(bi.ins, act_prev[0], sync=False,
                           reason="act-table-set-order")
        act_prev[0] = bi.ins

    for blk in range(nblk):
        b0 = blk * bpb

        # ---- load + transpose x/qk into xcatT [128=(fx|fqk), (i,tp)] ----
        # t-padded layout: each i-segment is [0,0,0, t0..t23] so the causal
        # conv folds into M1 as DC shifted matmuls.
        xcatT = sb.tile([128, CTP], FP, tag="xcatT")
        xcp3 = xcatT[:].rearrange("p (i t) -> p i t", t=TP)
        nc.vector.memset(xcp3[:, :, 0:DC - 1], 0.0)
        NG = 8                         # n's per transpose-psum group
        NCH = 32                       # n's per raw DMA chunk
        for c0 in range(0, bpb, BSUB):
            bs = min(BSUB, bpb - c0)
            nrow = bs * T
            slot = 64 if nrow <= 64 else 128   # bank-aligned transpose slots
            for src_d, half, tagr in ((x_d, 0, "xraw"), (qk_d, 1, "qraw")):
                for nh in range(0, n_c, NCH):
                    ncw = min(NCH, n_c - nh)
                    raw = sb.tile([nrow, NCH * F], FP, tag=tagr)
                    nc.sync.dma_start(
                        raw[:, :ncw * F],
                        src_d[b0 + c0:b0 + c0 + bs, :, nh:nh + ncw].rearrange(
                            "b t n f -> (b t) (n f)"),
                    )
                    for g in range((ncw + NG - 1) // NG):
                        ng = min(NG, ncw - g * NG)
                        pt = ps.tile([F, NG * slot], FP, tag="tps")
                        for k in range(ng):
                            nc.tensor.transpose(
                                pt[:, k * slot:k * slot + nrow],
                                raw[:, (g * NG + k) * F:(g * NG + k + 1) * F],
                                ident[:nrow, :nrow],
                            )
                        # psum [F, (n ng, b bs, t T)] -> xcatT (b*n_c+n), 3+t
                        dst = xcp3[half * F:(half + 1) * F, :, DC - 1:].rearrange(
                            "p (b n) t -> p n b t", b=bpb)[
                            :, nh + g * NG:nh + g * NG + ng, c0:c0 + bs]
                        src_ap = pt[:].rearrange(
                            "p (n r) -> p n r", r=slot)[:, :ng, :nrow].rearrange(
                            "p n (b t) -> p n b t", t=T)
                        nc.scalar.copy(out=dst, in_=src_ap)

        # ---- M1 (+folded conv): xconv = sum_k W1k[k].T @ shift(xcatT, k)
        #      z = W1z.T @ xcatT ; both silu'd straight out of PSUM ----
        xc2 = sb.tile([DI, CT], FP, tag="xc2")
        sz = sb2.tile([DI, CT], BF, tag="sz")
        W1k = [ct[f"W1k{k}"] for k in range(DC)]
        for ci in range(0, P, IML):
            c0 = ci * TP
            pxc = ps.tile([DI, CML], FP, tag="m1a")
            pz = ps.tile([DI, CML], FP, tag="m1b")
            for k in range(DC):
                sh = DC - 1 - k
                lo = max(0, sh - c0)   # only chunk 0 clips (pads absorb it)
                nc.tensor.matmul(pxc[:, lo:], W1k[k][:],
                                 xcatT[:, c0 + lo - sh:c0 + CML - sh],
                                 start=(k == 0), stop=(k == DC - 1))
            nc.tensor.matmul(pz[:], ct["W1z"][:], xcatT[:, c0:c0 + CML],
                             start=True, stop=True)
            # compact (drop pads) + silu, PSUM -> SBUF
            pxc3 = pxc[:].rearrange("p (i t) -> p i t", t=TP)
            pz3 = pz[:].rearrange("p (i t) -> p i t", t=TP)
            dxc = xc2[:, ci * T:(ci + IML) * T].rearrange(
                "p (i t) -> p i t", t=T)
            dsz = sz[:, ci * T:(ci + IML) * T].rearrange(
                "p (i t) -> p i t", t=T)
            chain(nc.scalar.activation(dxc, pxc3[:, :, DC - 1:], AF.Silu))
            chain(nc.scalar.activation(dsz, pz3[:, :, DC - 1:], AF.Silu))
        # xcD = xc2 * D (bf16, for gating later)
        xcD = sb2.tile([DI, CT], BF, tag="xcD")
        nc.scalar.activation(xcD[:], xc2[:], AF.Identity, scale=ct["D"][:, 0:1])

        # ---- M2: dt = softplus(W2dt.T @ xc2 + b_dt); bc = W2bc.T @ xc2 ----
        dt = sb.tile([DI, CT], FP, tag="dt")
        bc = sb.tile([2 * DS, CT], BF, tag="m2tmp")
        for c0 in range(0, CT, NMM):
            pdt = ps.tile([DI, NMM], FP, tag="m1a")
            pbc = ps.tile([2 * DS, NMM], FP, tag="m1b")
            nc.tensor.matmul(pdt[:], ct["W2dt"][:], xc2[:, c0:c0 + NMM],
                             start=True, stop=True)
            nc.tensor.matmul(pbc[:], ct["W2bc"][:], xc2[:, c0:c0 + NMM],
                             start=True, stop=True)
            # softplus(x + b_dt) = ln(1 + exp(x + b_dt)) (exp/ln share a set)
            spe = sb2.tile([DI, NMM], FP, tag="spe")
            chain(nc.scalar.activation(spe[:], pdt[:], AF.Exp,
                                       bias=ct["b_dt"][:, 0:1]))
            chain(nc.scalar.activation(dt[:, c0:c0 + NMM], spe[:], AF.Ln,
                                       bias=1.0))
            nc.scalar.copy(out=bc[:, c0:c0 + NMM], in_=pbc[:])

        # du = dt * xc2 (bf16 out; fp32 reads)
        duf = sb.tile([DI, CT], BF, tag="duf")
        nc.vector.tensor_tensor(duf[:], dt[:], xc2[:], OP.mult)

        # ---- transposes into scan layout [i, (d, t)] ----
        dtT = sb.tile([P, DI * T], FP, tag="dtT")
        duT = sb.tile([P, DI * T], BF, tag="duT")
        bcT = sb.tile([P, 2 * DS * T], BF, tag="bcT")
        for (srct, dstt, rows, idn, pdt) in (
                (dt, dtT, DI, ident, FP), (duf, duT, DI, ident_bf, BF),
                (bc, bcT, 2 * DS, ident_bf, BF)):
            s3 = srct[:].rearrange("p (i t) -> p i t", t=T)
            for t0 in range(0, T, TG):
                pt = ps.tile([P, TG * rows], pdt, tag="tps")
                for k in range(TG):
                    nc.tensor.transpose(
                        pt[:, k * rows:(k + 1) * rows],
                        s3[:rows, :, t0 + k],
                        idn[:rows, :rows],
                    )
                dst = dstt[:].rearrange("p (d t) -> p d t", t=T)[:, :, t0:t0 + TG]
                src = pt[:].rearrange("p (t d) -> p d t", t=TG)
                nc.scalar.copy(out=dst, in_=src)

        # ---- scan phase: one pass per state s over [i, (d=128, t=24)] ----
        duT3 = duT[:].rearrange("p (d t) -> p d t", t=T)
        bcT3 = bcT[:].rearrange("p (c t) -> p c t", t=T)
        ya = None
        for s in range(DS):
            dA = sb2.tile([P, DI * T], FP, tag="dA")
            chain(nc.scalar.activation(dA[:], dtT[:], AF.Exp, scale=A_s[s]))
            dA3 = dA[:].rearrange("p (d t) -> p d t", t=T)
            nc.gpsimd.memset(dA3[:, :, 0:1], 0.0)
            Xs = sb2.tile([P, DI * T], BF, tag="Xs")
            xeng.tensor_tensor(
                Xs[:].rearrange("p (d t) -> p d t", t=T),
                duT3[:, :, :],
                bcT3[:, s:s + 1, :].to_broadcast((P, DI, T)),
                OP.mult,
            )
            hs = sb2.tile([P, DI * T], BF, tag="hs")
            sp = nc.vector if s < n_dve_scan else scan_p
            sp.tensor_tensor_scan(hs[:], dA[:], Xs[:], 0.0, OP.mult, OP.add)
            tmp = sb2.tile([P, DI * T], BF, tag="Xs")
            ceng.tensor_tensor(
                tmp[:].rearrange("p (d t) -> p d t", t=T),
                hs[:].rearrange("p (d t) -> p d t", t=T),
                bcT3[:, DS + s:DS + s + 1, :].to_broadcast((P, DI, T)),
                OP.mult,
            )
            yb = sb2.tile([P, DI * T], BF, tag="yp")
            if ya is None:
                nc.vector.tensor_copy(out=yb[:], in_=tmp[:])
            else:
                aeng.tensor_tensor(yb[:], ya[:], tmp[:], OP.add)
            ya = yb

        # ---- transpose y back: [i,(d,t)] -> y_d [d,(i,t)] (bf16) ----
        y_d = sb.tile([DI, CT], BF, tag="y_d")
        ya3 = ya[:].rearrange("p (d t) -> p d t", t=T)
        for t0 in range(0, T, TG):
            pt = ps.tile([DI, TG * P], BF, tag="tps")
            for k in range(TG):
                nc.tensor.transpose(pt[:, k * P:(k + 1) * P],
                                    ya3[:, :, t0 + k], ident_bf[:P, :P])
            dst = y_d[:, :].rearrange(
                "p (i t) -> p i t", t=T)[:, :, t0:t0 + TG]
            nc.scalar.copy(out=dst,
                           in_=pt[:].rearrange("p (t i) -> p i t", t=TG))

        # ---- gate: y2 = (y_d + xcD) * sz  (bf16) ----
        t2 = sb.tile([DI, CT], BF, tag="xc")     # xc dead after conv
        nc.vector.tensor_tensor(t2[:], y_d[:], xcD[:], OP.add)
        y2 = sb.tile([DI, CT], BF, tag="duf")    # duf slot free now
        nc.vector.tensor_tensor(y2[:], t2[:], sz[:], OP.mult)

        # ---- M3: out = w_out.T @ y2 (bf16) ; transpose ; DMA out ----
        yo = sb.tile([F, CT], BF, tag="m2tmp")   # bc slot free now
        for c0 in range(0, CT, NMM):
            po = ps.tile([F, NMM], FP, tag="m1a")
            nc.tensor.matmul(po[:], w_out_bf[:], y2[:, c0:c0 + NMM],
                             start=True, stop=True)
            nc.scalar.copy(out=yo[:, c0:c0 + NMM], in_=po[:])
        yo4 = yo[:].rearrange("p (bl n t) -> p bl n t", n=n_c, t=T)
        TB = 4  # t's per out-transpose psum tile
        for bl in range(bpb):
            stg = sb.tile([n_c, T * F], FP, tag="ostg")
            for t0 in range(0, T, TB):
                pt = ps.tile([n_c, TB * F], BF, tag="tps")
                for k in range(TB):
                    nc.tensor.transpose(pt[:, k * F:(k + 1) * F],
                                        yo4[:, bl, :, t0 + k],
                                        ident_bf[:F, :F])
                nc.scalar.copy(out=stg[:, t0 * F:(t0 + TB) * F], in_=pt[:])
            nc.sync.dma_start(
                out_d[b0 + bl].rearrange("t n f -> n t f"),
                stg[:].rearrange("n (t f) -> n t f", f=F))
    es.close()


_CACHE = {}


def _get_program(key, consts, n_c, **kw):
    if key not in _CACHE:
        _CACHE[key] = build_program(n_c, consts, **kw)
    return _CACHE[key]


def _make_in_maps(inputs, consts):
    x = np.asarray(inputs["x"], np.float32)
    qk = np.asarray(inputs["qk"], np.float32)
    n_c = N // NCORES
    base = {
        "W1z": np.ascontiguousarray(consts["W1z"]),
        "W1k0": consts["W1xk"][0], "W1k1": consts["W1xk"][1],
        "W1k2": consts["W1xk"][2], "W1k3": consts["W1xk"][3],
        "W2dt": np.ascontiguousarray(consts["W2dt"]),
        "W2bc": np.ascontiguousarray(consts["W2bc"]),
        "b_dt": consts["b_dt"].reshape(DI, 1).copy(),
        "D": consts["D"].reshape(DI, 1).copy(),
        "w_out": np.ascontiguousarray(consts["w_out"]),
    }
    in_maps = []
    for c in range(NCORES):
        sl = slice(c * n_c, (c + 1) * n_c)
        m = dict(base)
        m["x_sh"] = np.ascontiguousarray(x[:, :, sl, :])
        m["qk_sh"] = np.ascontiguousarray(qk[:, :, sl, :])
        in_maps.append(m)
    return in_maps


def kernel(**inputs):
    consts = _host_consts(inputs)
    n_c = N // NCORES
    nc = _get_program("main", consts, n_c)
    in_maps = _make_in_maps(inputs, consts)
    res = run_bass_kernel_spmd(nc, in_maps, core_ids=list(range(NCORES)))
    out = np.empty((B, T, N, F), np.float32)
    for c in range(NCORES):
        sl = slice(c * n_c, (c + 1) * n_c)
        out[:, :, sl, :] = res.results[c]["out_sh"].reshape(B, T, n_c, F)
    return out


# revision 32
# speedup vs baseline: 1.0534x; 1.0534x over previous
"""Trainium2 Bass kernel for CustomMamba (data-parallel over (b*n) scans).

Self-contained: builds + compiles a single-core SPMD Bass/Tile program,
shards inputs over 8 NeuronCores (along n), runs via run_bass_kernel_spmd,
and gathers the full output.

v2: debug/asserts off, bf16 scan-phase elementwise (DVE 2x mode), scan op
on GpSimd (frees DVE), merged d-halves (DH=128), activation table-set
batching (Silu + natural_log_exp only), no DRAM spills, batched out-DMA.
"""

import sys

sys.path.insert(0, "/opt/trn_rl_repo")

import os

os.environ.setdefault("JAX_PLATFORMS", "cpu")

from contextlib import ExitStack

import numpy as np

import concourse.bacc as bacc
import concourse.mybir as mybir
from concourse.bass_utils import run_bass_kernel_spmd
from concourse.masks import make_identity
from concourse.tile import TileContext
from concourse.tile_rust import add_dep_helper

FP = mybir.dt.float32
BF = mybir.dt.bfloat16
AF = mybir.ActivationFunctionType
OP = mybir.AluOpType

# Problem constants (hardcoded per spec)
B, T, N, F = 8, 24, 512, 64
DI, DS, DR, DC = 128, 16, 4, 4
NCORES = 8


def _host_consts(inputs):
    """Fold the linear layers into per-stage weight matrices (fp32 numpy)."""
    w_mix = np.asarray(inputs["w_mix"], np.float32)      # [2F, F]
    b_mix = np.asarray(inputs["b_mix"], np.float32)      # [F]
    w_in = np.asarray(inputs["w_in"], np.float32)        # [F, 2*DI]
    conv_w = np.asarray(inputs["conv_w"], np.float32)    # [DI, DC]
    conv_b = np.asarray(inputs["conv_b"], np.float32)    # [DI]
    w_xproj = np.asarray(inputs["w_xproj"], np.float32)  # [DI, DR+2*DS]
    w_dt = np.asarray(inputs["w_dt"], np.float32)        # [DR, DI]
    b_dt = np.asarray(inputs["b_dt"], np.float32)        # [DI]
    A_log = np.asarray(inputs["A_log"], np.float32)      # [DI, DS]
    D = np.asarray(inputs["D"], np.float32)              # [DI]
    w_out = np.asarray(inputs["w_out"], np.float32)      # [DI, F]

    W1 = w_mix @ w_in                                    # [128, 2*DI]
    b1 = b_mix @ w_in                                    # [2*DI]
    W1x, W1z = W1[:, :DI].copy(), W1[:, DI:].copy()
    b1x, b1z = b1[:DI].copy(), b1[DI:].copy()
    # conv folded into M1: W1xk[k] = W1x * conv_w[:, k] (per-output-column)
    W1xk = [np.ascontiguousarray(W1x * conv_w[None, :, k]) for k in range(DC)]

    W2dt = (w_xproj[:, :DR] @ w_dt).copy()               # [DI, DI]
    W2bc = w_xproj[:, DR:].copy()                        # [DI, 2*DS]

    A = -np.exp(A_log)                                   # [DI, DS]
    assert np.allclose(A, A[0:1, :], rtol=1e-6), "A varies across d"
    A_s = [float(A[0, s]) for s in range(DS)]

    return dict(
        W1x=W1x, W1z=W1z, b1x=b1x, b1z=b1z, W1xk=W1xk,
        W2dt=W2dt, W2bc=W2bc, b_dt=b_dt,
        conv_w=conv_w, conv_b=conv_b, D=D, w_out=w_out, A_s=A_s,
    )


def build_program(n_c, consts, scan_eng="vector", x_eng="gpsimd",
                  cmul_eng="vector", add_eng="vector", n_dve_scan=0):
    """Build + compile the per-core Bass program. n_c = n-shard width."""
    nc = bacc.Bacc(
        "TRN2",
        target_bir_lowering=False,
        debug=False,
        enable_asserts=False,
        num_devices=1,
    )

    bn = B * n_c
    ic = min(128, bn)
    nblk = bn // ic
    assert nblk * ic == bn
    bpb = ic // n_c                    # b's per block
    assert bpb * n_c == ic and bpb >= 1
    CT = ic * T

    x_d = nc.dram_tensor("x_sh", (B, T, n_c, F), FP, kind="ExternalInput").ap()
    qk_d = nc.dram_tensor("qk_sh", (B, T, n_c, F), FP, kind="ExternalInput").ap()
    cd = {}
    for nm, shp in [
        ("W1z", (2 * F, DI)),
        ("W1k0", (2 * F, DI)), ("W1k1", (2 * F, DI)),
        ("W1k2", (2 * F, DI)), ("W1k3", (2 * F, DI)),
        ("W2dt", (DI, DI)), ("W2bc", (DI, 2 * DS)), ("b_dt", (DI, 1)),
        ("D", (DI, 1)), ("w_out", (DI, F)),
    ]:
        cd[nm] = nc.dram_tensor(nm, shp, FP, kind="ExternalInput").ap()
    out_d = nc.dram_tensor("out_sh", (B, T, n_c, F), FP, kind="ExternalOutput").ap()

    with TileContext(nc) as tc:
        _body(nc, tc, x_d, qk_d, cd, out_d,
              n_c, ic, nblk, bpb, CT, consts,
              scan_eng, x_eng, cmul_eng, add_eng, n_dve_scan)
    nc.compile()
    return nc


def _body(nc, tc, x_d, qk_d, cd, out_d,
          n_c, ic, nblk, bpb, CT, consts,
          scan_eng, x_eng, cmul_eng, add_eng, n_dve_scan):
    P = ic
    NMM = 512 if CT % 512 == 0 else CT  # matmul N-chunk
    TG = 4                              # t's merged per transpose-psum tile
    use_b1 = not (np.allclose(consts["b1x"], 0) and np.allclose(consts["b1z"], 0))
    use_cb = not np.allclose(consts["conv_b"], 0)
    A_s = consts["A_s"]

    es = ExitStack()
    sb = es.enter_context(tc.tile_pool(name="sb", bufs=1))
    sb2 = es.enter_context(tc.tile_pool(name="sb2", bufs=2))
    ps = es.enter_context(tc.tile_pool(name="ps", bufs=2, space="PSUM"))

    # ---- constants ----
    ct = {}
    for nm in cd:
        t = sb.tile(list(cd[nm].shape), FP, tag=f"c_{nm}")
        nc.sync.dma_start(t[:], cd[nm])
        ct[nm] = t
    w_out_bf = sb.tile([DI, F], BF, tag="c_w_out_bf")
    nc.vector.tensor_copy(out=w_out_bf[:], in_=ct["w_out"][:])
    ident = sb.tile([128, 128], FP, tag="ident")
    make_identity(nc, ident[:])
    ident_bf = sb.tile([128, 128], BF, tag="ident_bf")
    nc.vector.tensor_copy(out=ident_bf[:], in_=ident[:])

    BSUB = min(bpb, 128 // T)          # b's per load/transpose chunk

    scan_p = nc.gpsimd if scan_eng == "gpsimd" else nc.vector
    xeng = nc.vector if x_eng == "vector" else nc.gpsimd
    ceng = nc.vector if cmul_eng == "vector" else nc.gpsimd
    aeng = nc.vector if add_eng == "vector" else nc.gpsimd

    assert not use_b1 and not use_cb, "conv-folded M1 assumes zero biases"
    TP = T + DC - 1                    # padded t-stride (27)
    CTP = P * TP
    IML = 16                           # i's per M1 chunk
    CML = IML * TP                     # M1 chunk cols (432)

    act_prev = [None]

    def chain(bi):
        """Force ACT instruction order for table-set batching."""
        if act_prev[0] is not None:
            add_dep_helper(bi.ins, act_prev[0], sync=False,
                           reason="act-table-set-order")
        act_prev[0] = bi.ins

    for blk in range(nblk):
        b0 = blk * bpb

        # ---- load + transpose x/qk into xcatT [128=(fx|fqk), (i,tp)] ----
        # t-padded layout: each i-segment is [0,0,0, t0..t23] so the causal
        # conv folds into M1 as DC shifted matmuls.
        xcatT = sb.tile([128, CTP], FP, tag="xcatT")
        xcp3 = xcatT[:].rearrange("p (i t) -> p i t", t=TP)
        nc.vector.memset(xcp3[:, :, 0:DC - 1], 0.0)
        NG = 8                         # n's per transpose-psum group
        NCH = 32                       # n's per raw DMA chunk
        for c0 in range(0, bpb, BSUB):
            bs = min(BSUB, bpb - c0)
            nrow = bs * T
            slot = 64 if nrow <= 64 else 128   # bank-aligned transpose slots
            for src_d, half, tagr in ((x_d, 0, "xraw"), (qk_d, 1, "qraw")):
                for nh in range(0, n_c, NCH):
                    ncw = min(NCH, n_c - nh)
                    raw = sb.tile([nrow, NCH * F], FP, tag=tagr)
                    nc.sync.dma_start(
                        raw[:, :ncw * F],
                        src_d[b0 + c0:b0 + c0 + bs, :, nh:nh + ncw].rearrange(
                            "b t n f -> (b t) (n f)"),
                    )
                    for g in range((ncw + NG - 1) // NG):
                        ng = min(NG, ncw - g * NG)
                        pt = ps.tile([F, NG * slot], FP, tag="tps")
                        for k in range(ng):
                            nc.tensor.transpose(
                                pt[:, k * slot:k * slot + nrow],
                                raw[:, (g * NG + k) * F:(g * NG + k + 1) * F],
                                ident[:nrow, :nrow],
                            )
                        # psum [F, (n ng, b bs, t T)] -> xcatT (b*n_c+n), 3+t
                        dst = xcp3[half * F:(half + 1) * F, :, DC - 1:].rearrange(
                            "p (b n) t -> p n b t", b=bpb)[
                            :, nh + g * NG:nh + g * NG + ng, c0:c0 + bs]
                        src_ap = pt[:].rearrange(
                            "p (n r) -> p n r", r=slot)[:, :ng, :nrow].rearrange(
                            "p n (b t) -> p n b t", t=T)
                        nc.scalar.copy(out=dst, in_=src_ap)

        # ---- M1 (+folded conv): xconv = sum_k W1k[k].T @ shift(xcatT, k)
        #      z = W1z.T @ xcatT ; both silu'd straight out of PSUM ----
        xc2 = sb.tile([DI, CT], FP, tag="xc2")
        sz = sb2.tile([DI, CT], BF, tag="sz")
        W1k = [ct[f"W1k{k}"] for k in range(DC)]
        for ci in range(0, P, IML):
            c0 = ci * TP
            pxc = ps.tile([DI, CML], FP, tag="m1a")
            pz = ps.tile([DI, CML], FP, tag="m1b")
            for k in range(DC):
                sh = DC - 1 - k
                lo = max(0, sh - c0)   # only chunk 0 clips (pads absorb it)
                nc.tensor.matmul(pxc[:, lo:], W1k[k][:],
                                 xcatT[:, c0 + lo - sh:c0 + CML - sh],
                                 start=(k == 0), stop=(k == DC - 1))
            nc.tensor.matmul(pz[:], ct["W1z"][:], xcatT[:, c0:c0 + CML],
                             start=True, stop=True)
            # compact (drop pads) + silu, PSUM -> SBUF
            pxc3 = pxc[:].rearrange("p (i t) -> p i t", t=TP)
            pz3 = pz[:].rearrange("p (i t) -> p i t", t=TP)
            dxc = xc2[:, ci * T:(ci + IML) * T].rearrange(
                "p (i t) -> p i t", t=T)
            dsz = sz[:, ci * T:(ci + IML) * T].rearrange(
                "p (i t) -> p i t", t=T)
            chain(nc.scalar.activation(dxc, pxc3[:, :, DC - 1:], AF.Silu))
            chain(nc.scalar.activation(dsz, pz3[:, :, DC - 1:], AF.Silu))
        # xcD = xc2 * D (bf16, for gating later)
        xcD = sb2.tile([DI, CT], BF, tag="xcD")
        nc.scalar.activation(xcD[:], xc2[:], AF.Identity, scale=ct["D"][:, 0:1])

        # ---- M2: dt = softplus(W2dt.T @ xc2 + b_dt); bc = W2bc.T @ xc2 ----
        # All 6 Exp ops first (into spe_full), then ONE Ln — avoids the
        # greedy act-table chooser reloading exp/ln sets per chunk.
        dt = sb.tile([DI, CT], FP, tag="dt")
        bc = sb.tile([2 * DS, CT], BF, tag="m2tmp")
        spe_full = sb.tile([DI, CT], FP, tag="xcatT")  # xcatT dead after M1
        for c0 in range(0, CT, NMM):
            pdt = ps.tile([DI, NMM], FP, tag="m1a")
            pbc = ps.tile([2 * DS, NMM], FP, tag="m1b")
            nc.tensor.matmul(pdt[:], ct["W2dt"][:], xc2[:, c0:c0 + NMM],
                             start=True, stop=True)
            nc.tensor.matmul(pbc[:], ct["W2bc"][:], xc2[:, c0:c0 + NMM],
                             start=True, stop=True)
            chain(nc.scalar.activation(spe_full[:, c0:c0 + NMM], pdt[:],
                                       AF.Exp, bias=ct["b_dt"][:, 0:1]))
            nc.scalar.copy(out=bc[:, c0:c0 + NMM], in_=pbc[:])
        chain(nc.scalar.activation(dt[:], spe_full[:], AF.Ln, bias=1.0))

        # du = dt * xc2 (bf16 out; fp32 reads)
        duf = sb.tile([DI, CT], BF, tag="duf")
        nc.vector.tensor_tensor(duf[:], dt[:], xc2[:], OP.mult)

        # ---- transposes into scan layout [i, (d, t)] ----
        dtT = sb2.tile([P, DI * T], FP, tag="dtT")
        duT = sb2.tile([P, DI * T], BF, tag="duT")
        bcT = sb2.tile([P, 2 * DS * T], BF, tag="bcT")
        for (srct, dstt, rows, idn, pdt) in (
                (dt, dtT, DI, ident, FP), (duf, duT, DI, ident_bf, BF),
                (bc, bcT, 2 * DS, ident_bf, BF)):
            s3 = srct[:].rearrange("p (i t) -> p i t", t=T)
            for t0 in range(0, T, TG):
                pt = ps.tile([P, TG * rows], pdt, tag="tps")
                for k in range(TG):
                    nc.tensor.transpose(
                        pt[:, k * rows:(k + 1) * rows],
                        s3[:rows, :, t0 + k],
                        idn[:rows, :rows],
                    )
                dst = dstt[:].rearrange("p (d t) -> p d t", t=T)[:, :, t0:t0 + TG]
                src = pt[:].rearrange("p (t d) -> p d t", t=TG)
                nc.scalar.copy(out=dst, in_=src)

        # ---- scan phase: one pass per state s over [i, (d=128, t=24)] ----
        duT3 = duT[:].rearrange("p (d t) -> p d t", t=T)
        bcT3 = bcT[:].rearrange("p (c t) -> p c t", t=T)
        ya = None
        for s in range(DS):
            dA = sb2.tile([P, DI * T], FP, tag="dA")
            chain(nc.scalar.activation(dA[:], dtT[:], AF.Exp, scale=A_s[s]))
            dA3 = dA[:].rearrange("p (d t) -> p d t", t=T)
            nc.gpsimd.memset(dA3[:, :, 0:1], 0.0)
            Xs = sb2.tile([P, DI * T], BF, tag="Xs")
            xeng.tensor_tensor(
                Xs[:].rearrange("p (d t) -> p d t", t=T),
                duT3[:, :, :],
                bcT3[:, s:s + 1, :].to_broadcast((P, DI, T)),
                OP.mult,
            )
            hs = sb2.tile([P, DI * T], BF, tag="hs")
            sp = nc.vector if s < n_dve_scan else scan_p
            sp.tensor_tensor_scan(hs[:], dA[:], Xs[:], 0.0, OP.mult, OP.add)
            tmp = sb2.tile([P, DI * T], BF, tag="Xs")
            ceng.tensor_tensor(
                tmp[:].rearrange("p (d t) -> p d t", t=T),
                hs[:].rearrange("p (d t) -> p d t", t=T),
                bcT3[:, DS + s:DS + s + 1, :].to_broadcast((P, DI, T)),
                OP.mult,
            )
            yb = sb2.tile([P, DI * T], BF, tag="yp")
            if ya is None:
                nc.vector.tensor_copy(out=yb[:], in_=tmp[:])
            else:
                aeng.tensor_tensor(yb[:], ya[:], tmp[:], OP.add)
            ya = yb

        # ---- transpose y back: [i,(d,t)] -> y_d [d,(i,t)] (bf16) ----
        y_d = sb.tile([DI, CT], BF, tag="y_d")
        ya3 = ya[:].rearrange("p (d t) -> p d t", t=T)
        for t0 in range(0, T, TG):
            pt = ps.tile([DI, TG * P], BF, tag="tps")
            for k in range(TG):
                nc.tensor.transpose(pt[:, k * P:(k + 1) * P],
                                    ya3[:, :, t0 + k], ident_bf[:P, :P])
            dst = y_d[:, :].rearrange(
                "p (i t) -> p i t", t=T)[:, :, t0:t0 + TG]
            nc.scalar.copy(out=dst,
                           in_=pt[:].rearrange("p (t i) -> p i t", t=TG))

        # ---- gate: y2 = (y_d + xcD) * sz  (bf16) ----
        t2 = sb.tile([DI, CT], BF, tag="dt")     # dt dead after transposes
        nc.vector.tensor_tensor(t2[:], y_d[:], xcD[:], OP.add)
        y2 = sb.tile([DI, CT], BF, tag="duf")    # duf slot free now
        nc.vector.tensor_tensor(y2[:], t2[:], sz[:], OP.mult)

        # ---- M3: out = w_out.T @ y2 (bf16) ; transpose ; DMA out ----
        yo = sb.tile([F, CT], BF, tag="m2tmp")   # bc slot free now
        for c0 in range(0, CT, NMM):
            po = ps.tile([F, NMM], FP, tag="m1a")
            nc.tensor.matmul(po[:], w_out_bf[:], y2[:, c0:c0 + NMM],
                             start=True, stop=True)
            nc.scalar.copy(out=yo[:, c0:c0 + NMM], in_=po[:])
        yo4 = yo[:].rearrange("p (bl n t) -> p bl n t", n=n_c, t=T)
        TB = 4  # t's per out-transpose psum tile
        for bl in range(bpb):
            stg = sb.tile([n_c, T * F], FP, tag="ostg")
            for t0 in range(0, T, TB):
                pt = ps.tile([n_c, TB * F], BF, tag="tps")
                for k in range(TB):
                    nc.tensor.transpose(pt[:, k * F:(k + 1) * F],
                                        yo4[:, bl, :, t0 + k],
                                        ident_bf[:F, :F])
                nc.scalar.copy(out=stg[:, t0 * F:(t0 + TB) * F], in_=pt[:])
            nc.sync.dma_start(
                out_d[b0 + bl].rearrange("t n f -> n t f"),
                stg[:].rearrange("n (t f) -> n t f", f=F))
    es.close()


_CACHE = {}


def _get_program(key, consts, n_c, **kw):
    if key not in _CACHE:
        _CACHE[key] = build_program(n_c, consts, **kw)
    return _CACHE[key]


def _make_in_maps(inputs, consts):
    x = np.asarray(inputs["x"], np.float32)
    qk = np.asarray(inputs["qk"], np.float32)
    n_c = N // NCORES
    base = {
        "W1z": np.ascontiguousarray(consts["W1z"]),
        "W1k0": consts["W1xk"][0], "W1k1": consts["W1xk"][1],
        "W1k2": consts["W1xk"][2], "W1k3": consts["W1xk"][3],
        "W2dt": np.ascontiguousarray(consts["W2dt"]),
        "W2bc": np.ascontiguousarray(consts["W2bc"]),
        "b_dt": consts["b_dt"].reshape(DI, 1).copy(),
        "D": consts["D"].reshape(DI, 1).copy(),
        "w_out": np.ascontiguousarray(consts["w_out"]),
    }
    in_maps = []
    for c in range(NCORES):
        sl = slice(c * n_c, (c + 1) * n_c)
        m = dict(base)
        m["x_sh"] = np.ascontiguousarray(x[:, :, sl, :])
        m["qk_sh"] = np.ascontiguousarray(qk[:, :, sl, :])
        in_maps.append(m)
    return in_maps


def kernel(**inputs):
    consts = _host_consts(inputs)
    n_c = N // NCORES
    nc = _get_program("main", consts, n_c)
    in_maps = _make_in_maps(inputs, consts)
    res = run_bass_kernel_spmd(nc, in_maps, core_ids=list(range(NCORES)))
    out = np.empty((B, T, N, F), np.float32)
    for c in range(NCORES):
        sl = slice(c * n_c, (c + 1) * n_c)
        out[:, :, sl, :] = res.results[c]["out_sh"].reshape(B, T, n_c, F)
    return out
